# revision 13
# baseline (speedup 1.0000x reference)
"""AugmentedGeneEmbedding kernel for 8 TRN2 NeuronCores (Bass/Tile).

Math (per token t with gene g = idx[t]):
    id_vec  = id_table[g]                                  # [128]
    e       = gene_idx_to_esm_idx[g]
    valid   = (g < N_GENES) & (0 < e < V_ESM)
    seq     = valid ? esm_table[e] @ Wp + bp : 0           # [256]
    h       = concat([id_vec, tanh(gate) * seq])           # [384]
    y       = gelu(h @ W1 + b1) @ W2 + b2                  # [128]

Every factor depends only on the gene, so y[t] = Y[g(t)] for a per-gene
table Y.  The kernel therefore dedups tokens to unique genes:

  Phase A (per gene, ~2500/core): fused z = id @ W1_top
      + esm_row @ (tanh(g) Wp W1_bot) + mask * (tanh(g) bp W1_bot) + b1,
      Y = gelu(z) @ W2 + b2, accumulated into an SBUF-resident table.
  Phase B (per token): SBUF-source transpose dma_gather expands Y rows to
      tokens (SBUF->SBUF, no HBM round trip); the result is written
      feature-major ([128, n_tok], fully contiguous per partition) and the
      host folds the transpose into the shard-reassembly scatter.

Sharding: unique genes are snake-assigned to the 8 cores balancing token
counts; each core processes the tokens of its own genes.  Tables
replicated; all gathers on device.
"""

import numpy as np
import ml_dtypes

N_CORES = 8
B, K = 32, 2048
N_GENES, ID_DIM, ESM_DIM, PROJ, V_ESM = 20000, 128, 1280, 256, 30000
NTOK_TOTAL = B * K

NG_CAP = 2560   # unique-gene capacity per core (ceil(20000/8)=2500 padded to 5*512)
GT = 512        # genes per tile (esm gather + matmul chunk)
BIG = 2048      # tokens per phase-B gather batch (>1024 needs single_packet=False)

BF16 = ml_dtypes.bfloat16

_BUILD_CACHE = {}


def build_nc(n_tok):
    """Per-core Bass program (SPMD: same program on all 8 cores).
    n_tok = padded token capacity per core (multiple of 512)."""
    import concourse.bacc as bacc
    import concourse.mybir as mybir
    import concourse.tile as tile
    from contextlib import ExitStack

    fp32 = mybir.dt.float32
    bf16 = mybir.dt.bfloat16
    i16 = mybir.dt.int16
    AF = mybir.ActivationFunctionType

    assert n_tok % 512 == 0
    n_gt = NG_CAP // GT

    nc = bacc.Bacc("TRN2", target_bir_lowering=False, num_swdge_queues=4)

    eidx_d = nc.declare_dram_parameter("eidx16", [128, NG_CAP // 16], i16, isOutput=False)
    idid_d = nc.declare_dram_parameter("idid16", [128, NG_CAP // 16], i16, isOutput=False)
    tidx_d = nc.declare_dram_parameter("tidx16", [128, n_tok // 16], i16, isOutput=False)
    mask_d = nc.declare_dram_parameter("maskbf", [1, NG_CAP], bf16, isOutput=False)
    esm_d = nc.declare_dram_parameter("esmbf", [V_ESM + 1, ESM_DIM], bf16, isOutput=False)
    id_d = nc.declare_dram_parameter("idbf", [N_GENES, ID_DIM], bf16, isOutput=False)
    wp_d = nc.declare_dram_parameter("wpbf", [ESM_DIM, PROJ], bf16, isOutput=False)
    w1_d = nc.declare_dram_parameter("w1bf", [ID_DIM + PROJ, PROJ], bf16, isOutput=False)
    w2_d = nc.declare_dram_parameter("w2bf", [PROJ, ID_DIM], bf16, isOutput=False)
    bp_d = nc.declare_dram_parameter("bpw", [128, 2], bf16, isOutput=False)
    b1_d = nc.declare_dram_parameter("b1w", [128, 2], fp32, isOutput=False)
    b2_d = nc.declare_dram_parameter("b2row", [1, 128], fp32, isOutput=False)
    gate_d = nc.declare_dram_parameter("gatef", [1, 1], fp32, isOutput=False)
    # out is written feature-major: flat[p * n_tok + t] = y[t, feature p]
    out_d = nc.declare_dram_parameter("out", [n_tok, 128], bf16, isOutput=True)

    with tile.TileContext(nc) as tc, ExitStack() as ctx:
        const = ctx.enter_context(tc.tile_pool(name="const", bufs=1))
        idp = ctx.enter_context(tc.tile_pool(name="idgat", bufs=5))
        gpool = ctx.enter_context(tc.tile_pool(name="gather", bufs=n_gt))
        apool = ctx.enter_context(tc.tile_pool(name="act", bufs=4))
        opool = ctx.enter_context(tc.tile_pool(name="tokout", bufs=4))
        zps = ctx.enter_context(tc.tile_pool(name="zps", bufs=3, space="PSUM"))
        yps = ctx.enter_context(tc.tile_pool(name="yps", bufs=3, space="PSUM"))

        # ---------- constants ----------
        # Pool-engine DMA insts rotate through 8 global DMASW sems in the
        # scheduler's emission order, and each sem is locked to one SWDGE
        # queue.  Emission order is not source-controllable, so every gather
        # uses queue 0 (always self-consistent).
        # Dummy 16-idx gather issued first so the auto-inserted GPSIMD library
        # swap (a barrier on the gpsimd queue) happens during the NEFF preamble
        # instead of after all const DMAs.
        dummy_idx = const.tile([128, 1], i16)
        nc.vector.memset(dummy_idx[:], 0)
        dummy_out = const.tile([128, 1, 128], bf16)
        nc.gpsimd.dma_gather(dummy_out[:], esm_d[:, 0:128], dummy_idx[:], 16, 16, 128,
                             elem_step=ESM_DIM, queue_num=0)

        # Index tiles load on the scalar HWDGE queue so the gathers (gpsimd)
        # can start immediately; fold-critical weight loads go on sync.
        eidx_sb = const.tile([128, NG_CAP // 16], i16)
        nc.scalar.dma_start(eidx_sb[:], eidx_d[:])
        idid_sb = const.tile([128, NG_CAP // 16], i16)
        nc.scalar.dma_start(idid_sb[:], idid_d[:])
        tidx_sb = const.tile([128, n_tok // 16], i16)
        nc.scalar.dma_start(tidx_sb[:], tidx_d[:])

        # Gathers for the whole gene table issued up front; ring backpressure
        # paces them but nothing downstream needs gpsimd until phase B.
        gtiles = []
        itiles = []
        for g in range(n_gt):
            ic = g * (GT // 16)
            gtile = gpool.tile([128, 10, GT], bf16, tag="G", name=f"G{g}")
            nc.gpsimd.dma_gather(gtile[:], esm_d[:],
                                 eidx_sb[:, ic:ic + GT // 16], GT, GT, ESM_DIM,
                                 transpose=True, queue_num=0)
            gtiles.append(gtile)
            itile = idp.tile([128, 1, GT], bf16, tag="I", name=f"I{g}")
            nc.gpsimd.dma_gather(itile[:], id_d[:],
                                 idid_sb[:, ic:ic + GT // 16], GT, GT, ID_DIM,
                                 transpose=True, queue_num=0)
            itiles.append(itile)

        # Fold-critical weight loads after gather issuance in program order so
        # the GPSIMD library swap does not inherit their DMA-completion waits.
        wpT0 = const.tile([128, ESM_DIM], bf16)
        nc.sync.dma_start(wpT0[:], wp_d[:, 0:128], transpose=True)
        wpT1 = const.tile([128, ESM_DIM], bf16)
        nc.sync.dma_start(wpT1[:], wp_d[:, 128:256], transpose=True)
        w1b_sb = const.tile([128, 2, PROJ], bf16)
        nc.sync.dma_start(w1b_sb[:], w1_d[128:384, :].rearrange("(c p) f -> p c f", p=128))
        gate_sb = const.tile([1, 1], fp32)
        nc.sync.dma_start(gate_sb[:], gate_d[:])
        w1t_sb = const.tile([128, PROJ], bf16)
        nc.sync.dma_start(w1t_sb[:], w1_d[0:128, :])
        mask_sb = const.tile([1, NG_CAP], bf16)
        nc.scalar.dma_start(mask_sb[:], mask_d[:])
        w2_sb = const.tile([128, 2, 128], bf16)
        nc.scalar.dma_start(w2_sb[:], w2_d[:].rearrange("(c p) f -> p c f", p=128))
        bp_sb = const.tile([128, 2], bf16)
        nc.sync.dma_start(bp_sb[:], bp_d[:])
        b1_sb = const.tile([128, 2], fp32)
        nc.scalar.dma_start(b1_sb[:], b1_d[:])
        b2_sb = const.tile([1, 128], fp32)
        nc.scalar.dma_start(b2_sb[:], b2_d[:])

        ones1 = const.tile([1, 128], fp32)
        nc.vector.memset(ones1[:], 1.0)

        # Y table, SBUF-resident: gene slot s = r*128 + p lives at partition p,
        # rank r (256 B per rank) -- exactly the sbuf_tokens_per_rank=128
        # layout dma_gather's SBUF-source mode expects.
        y_sb = const.tile([128, NG_CAP // 128, 128], bf16)

        # ---------- one-time weight folding ----------
        tg_sb = const.tile([128, 1], fp32)         # tanh(gate) on every partition
        b2b_sb = const.tile([128, 128], fp32)      # b2 broadcast across partitions
        wc_sb = const.tile([128, 10, PROJ], bf16)  # Wc = tanh(g) * (Wp @ W1_bot)
        cb_sb = const.tile([1, PROJ], bf16)        # cb = tanh(g) * (bp @ W1_bot)
        with tc.tile_pool(name="foldps", bufs=2, space="PSUM") as fps:
            gb_ps = fps.tile([128, 1], fp32, tag="fold")
            nc.tensor.matmul(gb_ps[:], ones1[:], gate_sb[:], start=True, stop=True)
            nc.scalar.activation(tg_sb[:], gb_ps[:], AF.Tanh)

            b2b_ps = fps.tile([128, 128], fp32, tag="fold")
            nc.tensor.matmul(b2b_ps[:], ones1[:], b2_sb[:], start=True, stop=True)
            nc.vector.tensor_copy(b2b_sb[:], b2b_ps[:])

            for c in range(10):
                wc_ps = fps.tile([128, PROJ], fp32, tag="fold")
                nc.tensor.matmul(wc_ps[:], wpT0[:, c * 128:(c + 1) * 128],
                                 w1b_sb[:, 0, :], start=True, stop=False)
                nc.tensor.matmul(wc_ps[:], wpT1[:, c * 128:(c + 1) * 128],
                                 w1b_sb[:, 1, :], start=False, stop=True)
                nc.scalar.activation(wc_sb[:, c, :], wc_ps[:], AF.Copy,
                                     scale=tg_sb[:, 0:1])

            cb_ps = fps.tile([1, PROJ], fp32, tag="fold")
            nc.tensor.matmul(cb_ps[:], bp_sb[:, 0:1], w1b_sb[:, 0, :], start=True, stop=False)
            nc.tensor.matmul(cb_ps[:], bp_sb[:, 1:2], w1b_sb[:, 1, :], start=False, stop=True)
            nc.scalar.activation(cb_sb[:], cb_ps[:], AF.Copy, scale=tg_sb[0:1, 0:1])

        # ---------- phase A: per-gene table Y ----------
        for g in range(n_gt):
            gtile = gtiles[g]
            g0 = g * GT
            a_tiles = []
            for h in range(2):
                hs = slice(h * 128, (h + 1) * 128)
                zp = zps.tile([128, GT], fp32, tag="z")
                nc.tensor.matmul(zp[:], w1t_sb[:, hs], itiles[g][:, 0, :],
                                 start=True, stop=False)
                for c in range(10):
                    nc.tensor.matmul(zp[:], wc_sb[:, c, hs],
                                     gtile[:, c, :],
                                     start=False, stop=False)
                nc.tensor.matmul(zp[:], cb_sb[0:1, hs],
                                 mask_sb[0:1, g0:g0 + GT],
                                 start=False, stop=True)
                at = apool.tile([128, GT], bf16, tag="a")
                nc.scalar.activation(at[:], zp[:], AF.Gelu, bias=b1_sb[:, h:h + 1])
                a_tiles.append(at)
            for q in range(GT // 128):
                qs = slice(q * 128, (q + 1) * 128)
                yp = yps.tile([128, 128], fp32, tag="yp")
                nc.tensor.matmul(yp[:], a_tiles[0][:, qs], w2_sb[:, 0, :],
                                 start=True, stop=False)
                nc.tensor.matmul(yp[:], a_tiles[1][:, qs], w2_sb[:, 1, :],
                                 start=False, stop=True)
                nc.vector.tensor_add(y_sb[:, g * (GT // 128) + q, :], yp[:], b2b_sb[:])

        # ---------- phase B: token expansion from SBUF Y ----------
        outT = out_d[:].rearrange("(p w) f -> p (w f)", p=128)  # [128, n_tok]
        off = 0
        while off < n_tok:
            sz = min(BIG, n_tok - off)
            ot = opool.tile([128, 1, sz], bf16, tag="o")
            nc.gpsimd.dma_gather(ot[:], y_sb[:],
                                 tidx_sb[:, off // 16:(off + sz) // 16], sz, sz, 128,
                                 transpose=True, queue_num=0,
                                 single_packet=sz <= 1024,
                                 sbuf_tokens_per_rank=128,
                                 sbuf_free_dim_per_rank=256)
            nc.sync.dma_start(outT[:, off:off + sz], ot[:, 0, :])
            off += sz

    nc.compile()
    return nc


def _wrap16(a16):
    """int16 [n] -> [128, n//16]: logical index i at [i % 16 (+16k), i // 16]."""
    w = a16.reshape(-1, 16).T
    return np.tile(w, (8, 1)).copy()


def prepare_host(idx, gene_idx_to_esm_idx, id_table, esm_table, Wp, bp, gate,
                 W1, b1, W2, b2, n_cores=N_CORES):
    """Index prep + dtype/layout marshalling.

    Returns (shared, per_core, tok_pos, n_tok_cap) where tok_pos[c] are the
    original flat token positions handled by core c, in the order the core
    emits them (sorted by Y slot).
    """
    idx_flat = np.asarray(idx).reshape(-1).astype(np.int64)
    gmap = np.asarray(gene_idx_to_esm_idx).astype(np.int64)
    g_clip = np.clip(idx_flat, 0, N_GENES - 1)
    oob = (idx_flat < 0) | (idx_flat >= N_GENES)
    # key encodes (id row, forced-invalid) so OOB tokens get mask=0 entries
    key = np.where(oob, g_clip + N_GENES, g_clip)
    uniq, inv = np.unique(key, return_inverse=True)
    U = len(uniq)
    cnt = np.bincount(inv, minlength=U)

    # snake-assign genes (sorted by token count desc) to cores; slot = round
    order = np.argsort(-cnt, kind="stable")
    k = np.arange(U)
    rnd = k // n_cores
    c = k % n_cores
    core_snake = np.where(rnd % 2 == 0, c, n_cores - 1 - c)
    core_of = np.empty(U, np.int64)
    slot_of = np.empty(U, np.int64)
    core_of[order] = core_snake
    slot_of[order] = rnd
    assert U == 0 or int(slot_of.max()) < NG_CAP

    urow = np.where(uniq >= N_GENES, uniq - N_GENES, uniq)   # id-table row
    ue = gmap[np.clip(urow, 0, N_GENES - 1)]
    uvalid = (uniq < N_GENES) & (ue > 0) & (ue < V_ESM)
    ueidx = np.where(uvalid, ue, V_ESM)                      # row V_ESM is zero pad

    eidx_core = np.full((n_cores, NG_CAP), V_ESM, np.int16)
    idid_core = np.zeros((n_cores, NG_CAP), np.int16)
    mask_core = np.zeros((n_cores, NG_CAP), BF16)
    eidx_core[core_of, slot_of] = ueidx.astype(np.int16)
    idid_core[core_of, slot_of] = urow.astype(np.int16)
    mask_core[core_of, slot_of] = uvalid.astype(BF16)

    # Y-table slot as seen by phase B's SBUF gather: gene with phase-A column
    # c = slot % GT in tile g = slot // GT sits at rank g*4 + c//128,
    # partition c%128, i.e. logical gather index rank*128 + c%128.
    sg = slot_of // GT
    sc = slot_of % GT
    ysl_of = (sg * (GT // 128) + sc // 128) * 128 + (sc % 128)

    tok_core = core_of[inv]
    tok_ysl = ysl_of[inv]
    tok_pos = []
    for cc in range(n_cores):
        pos = np.nonzero(tok_core == cc)[0]
        pos = pos[np.argsort(tok_ysl[pos], kind="stable")]
        tok_pos.append(pos)
    n_max = max(len(p) for p in tok_pos)
    n_tok_cap = max(512, -(-n_max // 512) * 512)

    shared = {
        "esmbf": np.concatenate(
            [np.asarray(esm_table).astype(BF16), np.zeros((1, ESM_DIM), BF16)], axis=0),
        "idbf": np.asarray(id_table).astype(BF16),
        "wpbf": np.asarray(Wp).astype(BF16),
        "w1bf": np.asarray(W1).astype(BF16),
        "w2bf": np.asarray(W2).astype(BF16),
        "bpw": np.asarray(bp).astype(BF16).reshape(2, 128).T.copy(),
        "b1w": np.asarray(b1).astype(np.float32).reshape(2, 128).T.copy(),
        "b2row": np.asarray(b2).astype(np.float32).reshape(1, 128).copy(),
        "gatef": np.asarray(gate).astype(np.float32).reshape(1, 1).copy(),
    }
    per_core = []
    for cc in range(n_cores):
        tl = np.zeros(n_tok_cap, np.int16)
        pos = tok_pos[cc]
        tl[:len(pos)] = tok_ysl[pos].astype(np.int16)
        per_core.append({
            "eidx16": _wrap16(eidx_core[cc]),
            "idid16": _wrap16(idid_core[cc]),
            "tidx16": _wrap16(tl),
            "maskbf": mask_core[cc].reshape(1, -1).copy(),
        })
    return shared, per_core, tok_pos, n_tok_cap


def kernel(idx, gene_idx_to_esm_idx, id_table, esm_table, Wp, bp, gate,
           W1, b1, W2, b2, _trace=False, **_run_kwargs):
    from concourse.bass_utils import run_bass_kernel_spmd

    shared, per_core, tok_pos, n_tok_cap = prepare_host(
        idx, gene_idx_to_esm_idx, id_table, esm_table, Wp, bp, gate, W1, b1, W2, b2)
    if n_tok_cap not in _BUILD_CACHE:
        _BUILD_CACHE[n_tok_cap] = build_nc(n_tok_cap)
    nc = _BUILD_CACHE[n_tok_cap]

    in_maps = [dict(shared, **pc) for pc in per_core]
    res = run_bass_kernel_spmd(nc, in_maps, list(range(N_CORES)), trace=_trace,
                               **_run_kwargs)
    sh = np.asarray(idx).shape
    out = np.empty((NTOK_TOTAL, ID_DIM), np.float32)
    for c in range(N_CORES):
        pos = tok_pos[c]
        # device wrote feature-major: flat[p * n_tok_cap + t]
        oT = np.asarray(res.results[c]["out"]).reshape(ID_DIM, n_tok_cap)
        out[pos] = oT[:, :len(pos)].T.astype(np.float32)
    out = out.reshape(sh[0], sh[1], ID_DIM)
    if _trace:
        return out, res
    return out


# revision 14
# speedup vs baseline: 1.2394x; 1.2394x over previous
"""AugmentedGeneEmbedding kernel for 8 TRN2 NeuronCores (Bass/Tile).

Math (per token t with gene g = idx[t]):
    id_vec  = id_table[g]                                  # [128]
    e       = gene_idx_to_esm_idx[g]
    valid   = (g < N_GENES) & (0 < e < V_ESM)
    seq     = valid ? esm_table[e] @ Wp + bp : 0           # [256]
    h       = concat([id_vec, tanh(gate) * seq])           # [384]
    y       = gelu(h @ W1 + b1) @ W2 + b2                  # [128]

Every factor depends only on the gene, so y[t] = Y[g(t)] for a per-gene
table Y.  The kernel therefore dedups tokens to unique genes:

  Phase A (per gene, ~2500/core): fused z = id @ W1_top
      + esm_row @ (tanh(g) Wp W1_bot) + mask * (tanh(g) bp W1_bot) + b1,
      Y = gelu(z) @ W2 + b2, written fp32 to a DRAM scratch table in
      p-major row order (contiguous 2 KB per partition per tile).
  Phase B (per token): non-transpose dma_gather of 512 B fp32 Y rows by
      token, fp32->bf16 cast on DVE, contiguous per-partition writes to
      out (the host unpermutes row order during shard reassembly).

Sharding: unique genes are snake-assigned to the 8 cores balancing token
counts; each core processes the tokens of its own genes.  Tables
replicated; all gathers on device.

SWDGE queue plan: Pool-engine DMAs rotate through 8 global DMASW sems in
scheduler-emission order and each sem is locked to one queue.  We build
once with queue 0, read the emitted sem rotation, rebuild with
queue = sem % 4 per gather, and verify; fall back to single-queue if the
second schedule shifted.
"""

import numpy as np
import ml_dtypes

N_CORES = 8
B, K = 32, 2048
N_GENES, ID_DIM, ESM_DIM, PROJ, V_ESM = 20000, 128, 1280, 256, 30000
NTOK_TOTAL = B * K

NG_CAP = 2560   # unique-gene capacity per core (ceil(20000/8)=2500 padded to 5*512)
GT = 512        # genes per tile (esm gather + matmul chunk)
BIG = 4096      # tokens per phase-B gather batch (>1024 needs single_packet=False)

BF16 = ml_dtypes.bfloat16

_BUILD_CACHE = {}


def build_nc(n_tok, queue_plan=None):
    """Per-core Bass program (SPMD: same program on all 8 cores).
    n_tok = padded token capacity per core (multiple of 512).
    queue_plan maps gather source-index -> SWDGE queue (default all 0).
    Gather source order: esm g -> 2g, id g -> 2g+1, phase-B b -> 10+b."""
    import concourse.bacc as bacc
    import concourse.mybir as mybir
    import concourse.tile as tile
    from concourse import library_config
    from contextlib import ExitStack

    fp32 = mybir.dt.float32
    bf16 = mybir.dt.bfloat16
    i16 = mybir.dt.int16
    AF = mybir.ActivationFunctionType

    assert n_tok % 512 == 0
    n_gt = NG_CAP // GT
    qp = (queue_plan or {}).get

    nc = bacc.Bacc("TRN2", target_bir_lowering=False, num_swdge_queues=4)

    eidx_d = nc.declare_dram_parameter("eidx16", [128, NG_CAP // 16], i16, isOutput=False)
    idid_d = nc.declare_dram_parameter("idid16", [128, NG_CAP // 16], i16, isOutput=False)
    tidx_d = nc.declare_dram_parameter("tidx16", [128, n_tok // 16], i16, isOutput=False)
    mask_d = nc.declare_dram_parameter("maskbf", [1, NG_CAP], bf16, isOutput=False)
    esm_d = nc.declare_dram_parameter("esmbf", [V_ESM + 1, ESM_DIM], bf16, isOutput=False)
    id_d = nc.declare_dram_parameter("idbf", [N_GENES, ID_DIM], bf16, isOutput=False)
    wp_d = nc.declare_dram_parameter("wpbf", [ESM_DIM, PROJ], bf16, isOutput=False)
    w1_d = nc.declare_dram_parameter("w1bf", [ID_DIM + PROJ, PROJ], bf16, isOutput=False)
    w2_d = nc.declare_dram_parameter("w2bf", [PROJ, ID_DIM], bf16, isOutput=False)
    bp_d = nc.declare_dram_parameter("bpw", [128, 2], bf16, isOutput=False)
    b1_d = nc.declare_dram_parameter("b1w", [128, 2], fp32, isOutput=False)
    b2_d = nc.declare_dram_parameter("b2row", [1, 128], fp32, isOutput=False)
    gate_d = nc.declare_dram_parameter("gatef", [1, 1], fp32, isOutput=False)
    # out row p*(n_tok/128) + off/128 + c holds token (off + c*128 + p)
    out_d = nc.declare_dram_parameter("out", [n_tok, 128], bf16, isOutput=True)

    with tile.TileContext(nc) as tc, ExitStack() as ctx:
        const = ctx.enter_context(tc.tile_pool(name="const", bufs=1))
        idp = ctx.enter_context(tc.tile_pool(name="idgat", bufs=n_gt))
        gpool = ctx.enter_context(tc.tile_pool(name="gather", bufs=n_gt))
        apool = ctx.enter_context(tc.tile_pool(name="act", bufs=4))
        ypool = ctx.enter_context(tc.tile_pool(name="yout", bufs=3))
        opool = ctx.enter_context(tc.tile_pool(name="tokout", bufs=2))
        obp = ctx.enter_context(tc.tile_pool(name="tokoutb", bufs=2))
        dram = ctx.enter_context(tc.tile_pool(name="ydram", bufs=1, space="DRAM"))
        zps = ctx.enter_context(tc.tile_pool(name="zps", bufs=3, space="PSUM"))
        yps = ctx.enter_context(tc.tile_pool(name="yps", bufs=3, space="PSUM"))

        # Gather ucode library loaded explicitly up front so the swap barrier
        # runs during the NEFF preamble instead of gating on weight DMAs.
        nc.gpsimd.load_library(library_config.mlp)

        y_dram = dram.tile([NG_CAP, 128], fp32)
        # p-major view: row p*(NG_CAP/128) + r  <->  [p, r, f]
        y_pm = y_dram[:].rearrange("(p r) f -> p r f", p=128)

        # Index tiles load on the scalar HWDGE queue so the gathers (gpsimd)
        # can start immediately; fold-critical weight loads go on sync.
        eidx_sb = const.tile([128, NG_CAP // 16], i16)
        nc.scalar.dma_start(eidx_sb[:], eidx_d[:])
        idid_sb = const.tile([128, NG_CAP // 16], i16)
        nc.scalar.dma_start(idid_sb[:], idid_d[:])
        tidx_sb = const.tile([128, n_tok // 16], i16)
        nc.scalar.dma_start(tidx_sb[:], tidx_d[:])

        # Gathers for the whole gene table issued up front; ring backpressure
        # paces them but nothing downstream needs gpsimd until phase B.
        gtiles = []
        itiles = []
        for g in range(n_gt):
            ic = g * (GT // 16)
            gtile = gpool.tile([128, 10, GT], bf16, tag="G", name=f"G{g}")
            nc.gpsimd.dma_gather(gtile[:], esm_d[:],
                                 eidx_sb[:, ic:ic + GT // 16], GT, GT, ESM_DIM,
                                 transpose=True, queue_num=qp(2 * g, 0))
            gtiles.append(gtile)
            itile = idp.tile([128, 1, GT], bf16, tag="I", name=f"I{g}")
            nc.gpsimd.dma_gather(itile[:], id_d[:],
                                 idid_sb[:, ic:ic + GT // 16], GT, GT, ID_DIM,
                                 transpose=True, queue_num=qp(2 * g + 1, 0))
            itiles.append(itile)

        # Weight loads after gather issuance in program order.
        wpT0 = const.tile([128, ESM_DIM], bf16)
        nc.sync.dma_start(wpT0[:], wp_d[:, 0:128], transpose=True)
        wpT1 = const.tile([128, ESM_DIM], bf16)
        nc.sync.dma_start(wpT1[:], wp_d[:, 128:256], transpose=True)
        w1b_sb = const.tile([128, 2, PROJ], bf16)
        nc.sync.dma_start(w1b_sb[:], w1_d[128:384, :].rearrange("(c p) f -> p c f", p=128))
        gate_sb = const.tile([1, 1], fp32)
        nc.sync.dma_start(gate_sb[:], gate_d[:])
        w1t_sb = const.tile([128, PROJ], bf16)
        nc.sync.dma_start(w1t_sb[:], w1_d[0:128, :])
        mask_sb = const.tile([1, NG_CAP], bf16)
        nc.scalar.dma_start(mask_sb[:], mask_d[:])
        w2_sb = const.tile([128, 2, 128], bf16)
        nc.scalar.dma_start(w2_sb[:], w2_d[:].rearrange("(c p) f -> p c f", p=128))
        bp_sb = const.tile([128, 2], bf16)
        nc.sync.dma_start(bp_sb[:], bp_d[:])
        b1_sb = const.tile([128, 2], fp32)
        nc.scalar.dma_start(b1_sb[:], b1_d[:])
        b2_sb = const.tile([1, 128], fp32)
        nc.scalar.dma_start(b2_sb[:], b2_d[:])

        ones1 = const.tile([1, 128], fp32)
        nc.vector.memset(ones1[:], 1.0)

        # ---------- one-time weight folding ----------
        tg_sb = const.tile([128, 1], fp32)         # tanh(gate) on every partition
        b2b_sb = const.tile([128, 128], fp32)      # b2 broadcast across partitions
        wc_sb = const.tile([128, 10, PROJ], bf16)  # Wc = tanh(g) * (Wp @ W1_bot)
        cb_sb = const.tile([1, PROJ], bf16)        # cb = tanh(g) * (bp @ W1_bot)
        with tc.tile_pool(name="foldps", bufs=2, space="PSUM") as fps:
            gb_ps = fps.tile([128, 1], fp32, tag="fold")
            nc.tensor.matmul(gb_ps[:], ones1[:], gate_sb[:], start=True, stop=True)
            nc.scalar.activation(tg_sb[:], gb_ps[:], AF.Tanh)

            b2b_ps = fps.tile([128, 128], fp32, tag="fold")
            nc.tensor.matmul(b2b_ps[:], ones1[:], b2_sb[:], start=True, stop=True)
            nc.vector.tensor_copy(b2b_sb[:], b2b_ps[:])

            for c in range(10):
                wc_ps = fps.tile([128, PROJ], fp32, tag="fold")
                nc.tensor.matmul(wc_ps[:], wpT0[:, c * 128:(c + 1) * 128],
                                 w1b_sb[:, 0, :], start=True, stop=False)
                nc.tensor.matmul(wc_ps[:], wpT1[:, c * 128:(c + 1) * 128],
                                 w1b_sb[:, 1, :], start=False, stop=True)
                nc.scalar.activation(wc_sb[:, c, :], wc_ps[:], AF.Copy,
                                     scale=tg_sb[:, 0:1])

            cb_ps = fps.tile([1, PROJ], fp32, tag="fold")
            nc.tensor.matmul(cb_ps[:], bp_sb[:, 0:1], w1b_sb[:, 0, :], start=True, stop=False)
            nc.tensor.matmul(cb_ps[:], bp_sb[:, 1:2], w1b_sb[:, 1, :], start=False, stop=True)
            nc.scalar.activation(cb_sb[:], cb_ps[:], AF.Copy, scale=tg_sb[0:1, 0:1])

        # ---------- phase A: per-gene table Y ----------
        for g in range(n_gt):
            gtile = gtiles[g]
            g0 = g * GT
            a_tiles = []
            for h in range(2):
                hs = slice(h * 128, (h + 1) * 128)
                zp = zps.tile([128, GT], fp32, tag="z")
                nc.tensor.matmul(zp[:], w1t_sb[:, hs], itiles[g][:, 0, :],
                                 start=True, stop=False)
                for c in range(10):
                    nc.tensor.matmul(zp[:], wc_sb[:, c, hs],
                                     gtile[:, c, :],
                                     start=False, stop=False)
                nc.tensor.matmul(zp[:], cb_sb[0:1, hs],
                                 mask_sb[0:1, g0:g0 + GT],
                                 start=False, stop=True)
                at = apool.tile([128, GT], bf16, tag="a")
                nc.scalar.activation(at[:], zp[:], AF.Gelu, bias=b1_sb[:, h:h + 1])
                a_tiles.append(at)
            ysb = ypool.tile([128, GT // 128, 128], fp32, tag="y")
            for q in range(GT // 128):
                qs = slice(q * 128, (q + 1) * 128)
                yp = yps.tile([128, 128], fp32, tag="yp")
                nc.tensor.matmul(yp[:], a_tiles[0][:, qs], w2_sb[:, 0, :],
                                 start=True, stop=False)
                nc.tensor.matmul(yp[:], a_tiles[1][:, qs], w2_sb[:, 1, :],
                                 start=False, stop=True)
                nc.vector.tensor_add(ysb[:, q, :], yp[:], b2b_sb[:])
            # gene (g, q, p) -> Y row p*(NG_CAP/128) + g*4 + q  (2 KB/partition)
            nc.sync.dma_start(y_pm[:, g * (GT // 128):(g + 1) * (GT // 128), :], ysb[:])

        # ---------- phase B: token gather from Y ----------
        W = n_tok // 128
        outT = out_d[:].rearrange("(p w) f -> p (w f)", p=128)  # [128, n_tok]
        off = 0
        b = 0
        while off < n_tok:
            sz = min(BIG, n_tok - off)
            ot = opool.tile([128, sz // 128, 128], fp32, tag="o")
            nc.gpsimd.dma_gather(ot[:], y_dram[:],
                                 tidx_sb[:, off // 16:(off + sz) // 16], sz, sz, 128,
                                 elem_step=128, queue_num=qp(10 + b, 0),
                                 single_packet=sz <= 1024)
            ob = obp.tile([128, sz // 128, 128], bf16, tag="ob")
            nc.vector.tensor_copy(ob[:], ot[:])
            nc.sync.dma_start(outT[:, off:off + sz],
                              ob[:].rearrange("p a b -> p (a b)"))
            off += sz
            b += 1

    nc.compile()
    return nc


def _gather_emission(nc):
    """(source_sig, queue, sem_idx) per InstDMAGatherAnt in emission order."""
    import re
    import concourse.mybir as mybir
    out = []
    for i in nc.all_instructions():
        if type(i).__name__ != "InstDMAGatherAnt":
            continue
        sem = None
        if i.sync_info is not None:
            for u in i.sync_info.on_update:
                m = re.search(r"DMASW(\d+)_", str(u))
                if m:
                    sem = int(m.group(1))
        out.append((int(i.num_idxs), int(i.elem_size), bool(i.transpose),
                    int(i.queue_num), sem))
    return out


def _plan_queues(nc, n_tok):
    """Map gather source-index -> queue from the pass-1 sem rotation."""
    em = _gather_emission(nc)
    # expected source signatures in program order
    src = []
    for g in range(NG_CAP // GT):
        src.append((GT, ESM_DIM, True))    # 2g
        src.append((GT, ID_DIM, True))     # 2g+1
    off = 0
    while off < n_tok:
        sz = min(BIG, n_tok - off)
        src.append((sz, 128, False))       # 10+b
        off += sz
    if len(em) != len(src):
        return None
    # match instances by signature, relative order preserved
    from collections import defaultdict, deque
    pools = defaultdict(deque)
    for pos, (ni, es, tr, q, sem) in enumerate(em):
        pools[(ni, es, tr)].append(sem)
    plan = {}
    for si, sig in enumerate(src):
        if not pools[sig]:
            return None
        sem = pools[sig].popleft()
        if sem is None:
            return None
        plan[si] = sem % 4
    return plan


def _queues_consistent(nc):
    sems = {}
    for (ni, es, tr, q, sem) in _gather_emission(nc):
        if sem is None:
            return False
        if sems.setdefault(sem, q) != q:
            return False
    return True


def _build_best(n_tok):
    nc0 = build_nc(n_tok, None)
    try:
        plan = _plan_queues(nc0, n_tok)
        if plan and any(q != 0 for q in plan.values()):
            nc1 = build_nc(n_tok, plan)
            if _queues_consistent(nc1):
                return nc1
    except Exception:
        pass
    return nc0


def _wrap16(a16):
    """int16 [n] -> [128, n//16]: logical index i at [i % 16 (+16k), i // 16]."""
    w = a16.reshape(-1, 16).T
    return np.tile(w, (8, 1)).copy()


def prepare_host(idx, gene_idx_to_esm_idx, id_table, esm_table, Wp, bp, gate,
                 W1, b1, W2, b2, n_cores=N_CORES):
    """Index prep + dtype/layout marshalling.

    Returns (shared, per_core, tok_pos, n_tok_cap); tok_pos[c] are the
    original flat token positions handled by core c, in the order the core
    emits them (sorted by Y row).
    """
    idx_flat = np.asarray(idx).reshape(-1).astype(np.int64)
    gmap = np.asarray(gene_idx_to_esm_idx).astype(np.int64)
    g_clip = np.clip(idx_flat, 0, N_GENES - 1)
    oob = (idx_flat < 0) | (idx_flat >= N_GENES)
    # key encodes (id row, forced-invalid) so OOB tokens get mask=0 entries
    key = np.where(oob, g_clip + N_GENES, g_clip)
    uniq, inv = np.unique(key, return_inverse=True)
    U = len(uniq)
    cnt = np.bincount(inv, minlength=U)

    # snake-assign genes (sorted by token count desc) to cores; slot = round
    order = np.argsort(-cnt, kind="stable")
    k = np.arange(U)
    rnd = k // n_cores
    c = k % n_cores
    core_snake = np.where(rnd % 2 == 0, c, n_cores - 1 - c)
    core_of = np.empty(U, np.int64)
    slot_of = np.empty(U, np.int64)
    core_of[order] = core_snake
    slot_of[order] = rnd
    assert U == 0 or int(slot_of.max()) < NG_CAP

    urow = np.where(uniq >= N_GENES, uniq - N_GENES, uniq)   # id-table row
    ue = gmap[np.clip(urow, 0, N_GENES - 1)]
    uvalid = (uniq < N_GENES) & (ue > 0) & (ue < V_ESM)
    ueidx = np.where(uvalid, ue, V_ESM)                      # row V_ESM is zero pad

    eidx_core = np.full((n_cores, NG_CAP), V_ESM, np.int16)
    idid_core = np.zeros((n_cores, NG_CAP), np.int16)
    mask_core = np.zeros((n_cores, NG_CAP), BF16)
    eidx_core[core_of, slot_of] = ueidx.astype(np.int16)
    idid_core[core_of, slot_of] = urow.astype(np.int16)
    mask_core[core_of, slot_of] = uvalid.astype(BF16)

    # Y row in the p-major fp32 table: gene with phase-A column c = slot % GT
    # in tile g = slot // GT sits at row (c%128)*(NG_CAP/128) + g*4 + c//128.
    sg = slot_of // GT
    sc = slot_of % GT
    yrow_of = (sc % 128) * (NG_CAP // 128) + sg * (GT // 128) + sc // 128

    tok_core = core_of[inv]
    tok_yrow = yrow_of[inv]
    tok_pos = []
    for cc in range(n_cores):
        pos = np.nonzero(tok_core == cc)[0]
        pos = pos[np.argsort(tok_yrow[pos], kind="stable")]
        tok_pos.append(pos)
    n_max = max(len(p) for p in tok_pos)
    n_tok_cap = max(512, -(-n_max // 512) * 512)

    shared = {
        "esmbf": np.concatenate(
            [np.asarray(esm_table).astype(BF16), np.zeros((1, ESM_DIM), BF16)], axis=0),
        "idbf": np.asarray(id_table).astype(BF16),
        "wpbf": np.asarray(Wp).astype(BF16),
        "w1bf": np.asarray(W1).astype(BF16),
        "w2bf": np.asarray(W2).astype(BF16),
        "bpw": np.asarray(bp).astype(BF16).reshape(2, 128).T.copy(),
        "b1w": np.asarray(b1).astype(np.float32).reshape(2, 128).T.copy(),
        "b2row": np.asarray(b2).astype(np.float32).reshape(1, 128).copy(),
        "gatef": np.asarray(gate).astype(np.float32).reshape(1, 1).copy(),
    }
    per_core = []
    for cc in range(n_cores):
        tl = np.zeros(n_tok_cap, np.int16)
        pos = tok_pos[cc]
        tl[:len(pos)] = tok_yrow[pos].astype(np.int16)
        per_core.append({
            "eidx16": _wrap16(eidx_core[cc]),
            "idid16": _wrap16(idid_core[cc]),
            "tidx16": _wrap16(tl),
            "maskbf": mask_core[cc].reshape(1, -1).copy(),
        })
    return shared, per_core, tok_pos, n_tok_cap


def _dev_rows(n, n_tok_cap):
    """DRAM out row holding sorted-token position t (first n of n_tok_cap)."""
    t = np.arange(n)
    off = (t // BIG) * BIG
    r = t - off
    return (r % 128) * (n_tok_cap // 128) + off // 128 + r // 128


def kernel(idx, gene_idx_to_esm_idx, id_table, esm_table, Wp, bp, gate,
           W1, b1, W2, b2, _trace=False, **_run_kwargs):
    from concourse.bass_utils import run_bass_kernel_spmd

    shared, per_core, tok_pos, n_tok_cap = prepare_host(
        idx, gene_idx_to_esm_idx, id_table, esm_table, Wp, bp, gate, W1, b1, W2, b2)
    if n_tok_cap not in _BUILD_CACHE:
        _BUILD_CACHE[n_tok_cap] = _build_best(n_tok_cap)
    nc = _BUILD_CACHE[n_tok_cap]

    in_maps = [dict(shared, **pc) for pc in per_core]
    res = run_bass_kernel_spmd(nc, in_maps, list(range(N_CORES)), trace=_trace,
                               **_run_kwargs)
    sh = np.asarray(idx).shape
    out = np.empty((NTOK_TOTAL, ID_DIM), np.float32)
    for c in range(N_CORES):
        pos = tok_pos[c]
        rows = np.asarray(res.results[c]["out"])
        out[pos] = rows[_dev_rows(len(pos), n_tok_cap)].astype(np.float32)
    out = out.reshape(sh[0], sh[1], ID_DIM)
    if _trace:
        return out, res
    return out
